# revision 2
# baseline (speedup 1.0000x reference)
"""Conv3D (stride (1,2,2), pad (2,3,3)) as a Bass/Tile kernel for 8 trn2 cores.

Problem: x (8,3,16,112,112) f32, weight (64,3,5,7,7), bias (64,)
      -> out (8,64,16,56,56).  Data-parallel: one batch sample per core.

od-pair packing: adjacent output depths (2jp, 2jp+1) share 6 input depth
slices (kd window 5, depth stride 1), so the contraction rows
k = (c, dd in 0..6, kw) = 126 (+1 ones row for bias) ride the PE partition
axis while the stationary matrix holds BOTH depths x 64 channels in its
128 columns:

  w2[k=(c,dd,kw), kh, o]      = W[o, c, dd,   kh, kw]   (dd<5, od=2jp)
  w2[k=(c,dd,kw), kh, 64+o]   = W[o, c, dd-1, kh, kw]   (dd>0, od=2jp+1)

One shared moving stream (the kw-shifted, stride-2-W-selected rows of the
6-slice depth window) feeds the whole 128x128 array: K*M = 127*128 vs the
baseline's 106*64 -> ~2x fewer streamed cycles and half the input DMA.
kh = 7 is a PSUM accumulation loop via free-dim H offsets; per od-pair the
full 56x56 output plane lives in 7 PSUM tiles [128, 448] (7 banks).
"""

import numpy as np
import ml_dtypes

import concourse.bass as bass
import concourse.mybir as mybir
import concourse.tile as tile
from concourse import bacc
from concourse.bass_utils import run_bass_kernel_spmd

N, C, D, H, W = 8, 3, 16, 112, 112
O, KD, KH, KW = 64, 5, 7, 7
PD, PH, PW = 2, 3, 3
OD, OH, OW = 16, 56, 56
DD = KD + 1               # depth window per od-pair
KP = C * DD * KW          # 126 contraction rows
NP = KP + 1               # + ones row for bias
HP = H + 2 * PH           # 118 padded input rows
JP = OD // 2              # 8 od-pairs per sample
OHB = 8                   # output rows per matmul tile
T = OH // OHB             # 7 tiles per od-pair
NF = OHB * OW             # 448 moving free size per matmul

BF16 = mybir.dt.bfloat16

_CACHE = {}
LAST_RUN = None


def _build_bass():
    nc = bacc.Bacc("TRN2", target_bir_lowering=False, debug=False, num_devices=N)
    f32 = mybir.dt.float32
    r = nc.dram_tensor("r", [JP, NP, HP, OW], BF16, kind="ExternalInput")
    w = nc.dram_tensor("w", [NP, KH, 128], BF16, kind="ExternalInput")
    out = nc.dram_tensor("out", [128, JP, T, NF], BF16, kind="ExternalOutput")

    with tile.TileContext(nc) as tc:
        with (
            tc.tile_pool(name="wp", bufs=1) as wp,
            tc.tile_pool(name="sp", bufs=5) as sp,
            tc.tile_pool(name="op", bufs=3) as op,
            tc.tile_pool(name="pp", bufs=8, space=bass.MemorySpace.PSUM) as pp,
        ):
            # weight load first on the SWDGE queue: its packets lead every
            # engine FIFO (a scalar-queue HWDGE weight load saw its last
            # completion packet HOL-blocked 52us behind input-load packets).
            wt = wp.tile([NP, KH, 128], BF16)
            nc.gpsimd.dma_start(wt[:], w[:])

            # PE warm-up: matmuls on a memset-zeroed tile into scratch PSUM
            # (no DMA dependency at all), so the HAM clock-gate is at 8/8
            # and the PE queue is hot when the first real matmul is ready.
            wz = wp.tile([128, 512], BF16)
            nc.vector.memset(wz[:], 0.0)
            for _ in range(44):
                wu = pp.tile([128, NF], f32, name="ps")
                nc.tensor.matmul(wu[:, :], wz[0:NP, 0:128], wz[0:NP, 64:512],
                                 start=True, stop=True)

            cp = 0
            for j in range(JP):
                s = sp.tile([NP, HP, OW], BF16)
                # One dma_start lands on ONE SDMA engine (~22 GB/s HBM) -
                # split each chunk load so it spreads across engines; the
                # first chunks get the finest split to cut the preamble.
                nsp = 8 if j < 2 else 6
                bnd = [round(NP * i / nsp) for i in range(nsp + 1)]
                for a, b2 in zip(bnd, bnd[1:]):
                    nc.gpsimd.dma_start(s[a:b2], r[j, a:b2])
                ob = op.tile([128, T, NF], BF16)
                ps = [pp.tile([128, NF], f32, name="ps") for _ in range(T)]
                for kh in range(KH):
                    for t in range(T):
                        b = 16 * t + kh
                        nc.tensor.matmul(
                            ps[t][:, :], wt[0:NP, kh, :],
                            s[0:NP, b : b + 2 * OHB : 2, :],
                            start=(kh == 0), stop=(kh == KH - 1),
                        )
                for t in range(T):
                    if cp % 2 == 0:
                        nc.scalar.copy(ob[0:128, t], ps[t][:])
                    else:
                        nc.vector.tensor_copy(ob[0:128, t], ps[t][:])
                    cp += 1
                nc.sync.dma_start(out[0:128, j, 0:4], ob[0:128, 0:4])
                nc.sync.dma_start(out[0:128, j, 4:7], ob[0:128, 4:7])
    nc.compile()
    return nc


def _host_pack(x, weight, bias):
    """Pre-shifted rhs volume R per sample (bf16) and the packed weights."""
    xf = np.ascontiguousarray(x, dtype=np.float32)
    xp = np.zeros((N, C, D + 2 * PD, HP, W + 2 * PW), np.float32)
    xp[:, :, PD : PD + D, PH : PH + H, PW : PW + W] = xf

    # R[n, jp, k=(c,dd,kw), hp, ow] = xp[n, c, 2jp+dd, hp, 2*ow+kw]
    R = np.empty((N, JP, NP, HP, OW), np.float32)
    k = 0
    for c in range(C):
        for dd in range(DD):
            # depth slices 2jp+dd for jp in 0..7 -> xp[:, c, dd:dd+16:2]
            sl = xp[:, c, dd : dd + OD : 2]            # [N, JP, HP, 118]
            for kw in range(KW):
                R[:, :, k] = sl[:, :, :, kw : kw + 2 * OW : 2]
                k += 1
    R[:, :, KP] = 1.0
    Rb = R.astype(ml_dtypes.bfloat16)

    # w2[k=(c,dd,kw), kh, half*64+o]
    Wt = np.zeros((C, DD, KW, KH, 128), np.float32)
    wf = np.asarray(weight, np.float32).transpose(1, 2, 4, 3, 0)  # [C,KD,KW,KH,O]
    Wt[:, :KD, :, :, 0:O] = wf
    Wt[:, 1:, :, :, O:128] = wf
    Wt = Wt.reshape(KP, KH, 128)
    W2 = np.zeros((NP, KH, 128), np.float32)
    W2[:KP] = Wt
    W2[KP, 0, 0:O] = np.asarray(bias, np.float32)
    W2[KP, 0, O:128] = np.asarray(bias, np.float32)
    return Rb, W2.astype(ml_dtypes.bfloat16)


def kernel(x, weight, bias):
    global LAST_RUN
    if "nc" not in _CACHE:
        _CACHE["nc"] = _build_bass()
    nc = _CACHE["nc"]

    Rb, Wt = _host_pack(x, weight, bias)
    in_maps = [{"r": Rb[n], "w": Wt} for n in range(N)]
    res = run_bass_kernel_spmd(nc, in_maps, core_ids=list(range(N)))
    LAST_RUN = res
    outs = []
    for n in range(N):
        arr = np.asarray(res.results[n]["out"]).astype(np.float32)
        # [128, JP, T, NF] -> [half,o, jp, t, r, w] -> (O, OD, OH, OW)
        arr = arr.reshape(2, O, JP, T, OHB, OW)
        arr = arr.transpose(1, 2, 0, 3, 4, 5).reshape(O, OD, OH, OW)
        outs.append(arr)
    return np.stack(outs, axis=0)
